# revision 5
# baseline (speedup 1.0000x reference)
"""Fused linear + cross-entropy loss on 8 Trainium2 NeuronCores.

Vocab-parallel fp8 (e4m3) DoubleRow matmul; the per-core tensor-engine
stream is the whole story: 6283 vocab cols x (2048/256) k-pairs x 32
token tiles = 1.61M moving columns/core. Measured wall: with all 8
cores streaming fp8 DoubleRow matmuls back-to-back the sustained rate
is ~0.51 ns/col (one core alone: ~0.42 ns/col = the 2.4 GHz ideal), a
chip-level power/thermal downclock that no instruction restructuring
recovers: a matmul-only skeleton with no ACT/DVE/DMA and LDWEIGHTS
elided measures the same. So the kernel just (a) never streams padding
(vocab split 8x6283 = 50264, 7 pad rows, vs the naive 8x6400 = +1.8%),
and (b) keeps everything else off the critical path:

  - x^T [2048, 4096] fp8 resident in SBUF as 8 k-pair tiles
  - W^T shard [2048, 6283] fp8 streamed in groups [139, 1024 x 6]
  - per (group, token-tile): PSUM [128, <=1024] (2 banks, 4 bufs),
    j_outer so consecutive matmuls share the stationary x k-pair and
    LDWEIGHTS hides under the previous matmuls' streaming
  - ScalarE: exp(psum/64) + per-partition row-sum in ONE activation
    (accum_out) -> se_acc column; no per-tile vector work at all
  - picked-label logit: computed on the host exactly in fp32
    (x[t] . w[label_t]) instead of on-device masking - more accurate
    and removes 192 vector instructions

Host merge: sumexp per token = sum over cores/groups - 7, lse = log,
loss = mean(lse - picked) + 1e-4 * mean(lse^2).  W is pre-scaled by 64
(so fp8 e4m3 entries ~N(0,1) avoid subnormals); exp undoes it via the
activation scale. HW loss rel err ~6e-5 (gate: 2e-2).
"""

import numpy as np

import concourse.bass as bass
import concourse.mybir as mybir
import concourse.tile as tile
from concourse import bacc
from concourse.bass_utils import run_bass_kernel_spmd

D = 2048
V_TOTAL = 50257
N_CORES = 8
V_LOC = 6283                       # ceil(50257/8); 8*6283 = 50264
N_PAD = N_CORES * V_LOC - V_TOTAL  # 7 zero rows, on core 7
N_TOK = 4096
KT = D // 128                      # 16
G_SIZES = [139] + [1024] * 6       # sum = 6283; tiny group first so the
# per-repeat boundary prefetch is a 0.3MB tile (10us of DMA cover needed)
# instead of 2.1MB, and the first matmuls at cold start gate on less data
NG = len(G_SIZES)
IGNORE_INDEX = -100
Z_REG = 1e-4
W_SCALE = 64.0
F32 = mybir.dt.float32
FP8 = mybir.dt.float8e4
NP_FP8 = mybir.dt.np(FP8)


def build_nc(n_tok=N_TOK, repeats=1, psum_bufs=4, w_bufs=3, ex_bufs=4):
    t_tiles = n_tok // 128
    ncols = t_tiles * NG
    gmax = max(G_SIZES)

    nc = bacc.Bacc(None, target_bir_lowering=False, debug=False)

    xT = nc.dram_tensor("xT", [D, n_tok], FP8, kind="ExternalInput")
    wT = nc.dram_tensor("wT", [D, V_LOC], FP8, kind="ExternalInput")
    sumexp = nc.dram_tensor("sumexp", [128, ncols], F32, kind="ExternalOutput")

    xT_r = xT.rearrange("(k p) n -> p k n", p=128)   # [128, KT, n_tok]
    wT_r = wT.rearrange("(k p) v -> p k v", p=128)   # [128, KT, v_loc]

    with tile.TileContext(nc) as tc:
        with (
            tc.tile_pool(name="xpool", bufs=1) as xpool,
            tc.tile_pool(name="wpool", bufs=w_bufs) as wpool,
            tc.tile_pool(name="cpool", bufs=1) as cpool,
            tc.tile_pool(name="spool", bufs=ex_bufs) as spool,
            tc.tile_pool(name="ppool", bufs=psum_bufs, space=bass.MemorySpace.PSUM) as ppool,
        ):
            # group 0's W leads the DMA queue (small, gates first matmuls)
            w0_sb = wpool.tile([128, KT, gmax], FP8, tag="w")
            nc.sync.dma_start(out=w0_sb[:, :, : G_SIZES[0]], in_=wT_r[:, :, : G_SIZES[0]])

            x_pairs = []
            for j in range(KT // 2):
                xp = xpool.tile([128, 2, n_tok], FP8, tag=f"x{j}")
                nc.sync.dma_start(out=xp[:], in_=xT_r[:, 2 * j : 2 * j + 2, :])
                x_pairs.append(xp)

            se_acc = cpool.tile([128, ncols], F32)

            for _rep in range(repeats):
                off = 0
                for g, gs in enumerate(G_SIZES):
                    if g == 0 and _rep == 0:
                        w_sb = w0_sb
                    else:
                        w_sb = wpool.tile([128, KT, gmax], FP8, tag="w")
                        nc.sync.dma_start(
                            out=w_sb[:, :, :gs], in_=wT_r[:, :, off : off + gs]
                        )
                    nbank = (gs + 511) // 512
                    for t in range(t_tiles):
                        ps = ppool.tile([128, gmax], F32, tag="ps")
                        tok = slice(t * 128, (t + 1) * 128)
                        for j in range(KT // 2):
                            for b in range(nbank):
                                vs = min(512, gs - b * 512)
                                bank = slice(b * 512, b * 512 + vs)
                                nc.tensor.matmul(
                                    ps[:, bank],
                                    x_pairs[j][:, :, tok],
                                    w_sb[:, 2 * j : 2 * j + 2, bank],
                                    start=(j == 0),
                                    stop=(j == KT // 2 - 1),
                                    perf_mode=mybir.MatmulPerfMode.DoubleRow,
                                    skip_group_check=True,
                                )
                        col = t * NG + g
                        ex = spool.tile([128, gmax], F32, tag="ex")
                        nc.scalar.activation(
                            ex[:, :gs],
                            ps[:, :gs],
                            mybir.ActivationFunctionType.Exp,
                            scale=1.0 / W_SCALE,
                            accum_out=se_acc[:, col : col + 1],
                        )
                    off += gs

            nc.sync.dma_start(out=sumexp[:], in_=se_acc[:])

    nc.compile()
    return nc


def make_in_maps(x, labels, weight, n_cores=N_CORES):
    xf = np.ascontiguousarray(x.reshape(N_TOK, D).T).astype(NP_FP8)
    wb = (weight * np.float32(W_SCALE)).astype(NP_FP8)
    wpad = np.zeros((n_cores * V_LOC, D), NP_FP8)
    wpad[: weight.shape[0]] = wb

    in_maps = []
    for m in range(n_cores):
        wT_m = np.ascontiguousarray(wpad[m * V_LOC : (m + 1) * V_LOC].T)
        in_maps.append({"xT": xf, "wT": wT_m})
    return in_maps


def merge_results(results, x, labels, weight):
    se = np.stack([np.asarray(r["sumexp"], np.float64) for r in results])
    # [cores, 128, T*NG] -> per-token: token (t, p) = t*128 + p
    t_tiles = N_TOK // 128
    se_tok = se.reshape(-1, 128, t_tiles, NG).sum(3).transpose(0, 2, 1).reshape(-1, N_TOK)
    sumexp_tok = se_tok.sum(0) - float(N_PAD)

    lab_flat = np.asarray(labels).reshape(-1).astype(np.int64)
    valid = lab_flat != IGNORE_INDEX
    safe = np.where(valid, lab_flat, 0)
    xf = np.asarray(x, np.float32).reshape(N_TOK, D)
    w = np.asarray(weight, np.float32)
    picked = np.einsum("td,td->t", xf, w[safe], dtype=np.float64)

    n_valid = float(valid.sum())
    denom = max(n_valid, 1.0)
    lse = np.log(sumexp_tok)
    nll = lse - picked
    loss = np.where(valid, nll, 0.0).sum() / denom
    if Z_REG > 0.0 and n_valid > 0:
        loss = loss + Z_REG * np.where(valid, lse * lse, 0.0).sum() / denom
    return np.float32(loss)


_CACHE = {}


def kernel(x, labels, weight):
    x = np.asarray(x, dtype=np.float32)
    labels_np = np.asarray(labels)
    weight = np.asarray(weight, dtype=np.float32)

    if "nc" not in _CACHE:
        _CACHE["nc"] = build_nc()
    nc = _CACHE["nc"]

    in_maps = make_in_maps(x, labels_np, weight)
    res = run_bass_kernel_spmd(nc, in_maps, core_ids=list(range(N_CORES)))
    return merge_results(res.results, x, labels_np, weight)
